# revision 1
# baseline (speedup 1.0000x reference)
"""Channel-attention (CAM) Trainium2 kernel.

Reference computation (per batch b of 16):
    q   = x[b].reshape(C, HW)                  # C=512, HW=4096
    sim = q @ q.T                              # [C, C], symmetric
    attn = softmax(max(sim) - sim, axis=-1)    # == exp(min_r - sim) / Z_r
    out[b] = gamma * attn @ q + x[b]

Sharding: data-parallel over batch across 8 NeuronCores (2 batches/core).
kernel() takes full inputs, shards internally, returns the full output.

Per-core kernel design:
  - All matmuls in float32r (fp32 storage, ~13-bit mantissa, full PE rate
    at N>=256). bf16 is NOT accurate enough here: the softmax is
    winner-take-all (sim entries spread sigma~64), so sim errors ~0.3 flip
    argmin rows. f32r gives end-to-end max_abs_err ~0.018 (rel L2 4e-4).
  - x is streamed in column-waves, rounded to f32r (DVE), transposed on
    the PE (via identity matmuls) into qT tiles [n, c]; sim matmuls run
    two transpose-chunks behind, so DMA/round/transpose/matmul pipeline.
  - sim is symmetric: compute block rows over cols >= (0,128,256,256)
    only, and fill the 5 missing lower [128,128] blocks by PE-transposing
    their mirror blocks out of PSUM.
  - softmax via ACT: p = exp(min_r - sim) with accum_out producing Z in
    the same pass; rows scaled by gamma/Z (DVE), PE-transposed, and the
    identity is added to the diagonal so the second matmul directly
    computes gamma*attn@q + q = out (residual folded into the matmul;
    note x == q here).
  - mm2 results are staged [128, 2048] in SBUF and stored with 8KB
    descriptor lines, all on the Sync HWDGE queue.
  - phase-reordered emission: batch-1's load/transpose/sim phase is
    emitted BEFORE batch-0's second matmul, so mm2(0) becomes the
    lower-priority PE filler behind batch-1's dependency chains, and the
    Sync queue orders loads(0), loads(1), stores(0), stores(1) — store
    sem-waits never block loads.
  - 8 dummy identity matmuls at t=0 pre-warm the PE clock gate (HAM)
    while the first loads are in flight.

Measured on trn2 (8 cores, axon): ~163-169 us HW exec for the full B=16
problem (PE 95%-dense over its span), vs ~125 us warm-PE-roofline for
this decomposition plus ~7 us preamble and ~13 us tail drain.
"""
import sys

if "/opt/trn_rl_repo" not in sys.path:
    sys.path.insert(0, "/opt/trn_rl_repo")

import numpy as np

B, C, H, W = 16, 512, 64, 64
HW = H * W
NCORES = 8
NB = B // NCORES          # batches per core
P = 128
CB = C // P               # 4 channel blocks
KN = HW // P              # 32 contraction chunks for sim
NJ = HW // 512            # 8 output column chunks

_BUILD_CACHE = {}


def build_bass():
    import concourse.bacc as bacc
    import concourse.tile as tile
    from concourse import mybir
    from concourse.masks import make_identity

    f32 = mybir.dt.float32
    f32r = mybir.dt.float32r
    AX = mybir.AxisListType
    ALU = mybir.AluOpType
    ACTF = mybir.ActivationFunctionType

    nc = bacc.Bacc()
    x_ext = nc.declare_dram_parameter("x", [NB, C, HW], f32, isOutput=False)
    g_ext = nc.declare_dram_parameter("gamma", [1], f32, isOutput=False)
    o_ext = nc.declare_dram_parameter("out", [NB, C, HW], f32, isOutput=True)

    # alternate PSUM->SBUF copies between ACT and DVE to balance engines
    _flip = [0]

    with tile.TileContext(nc) as tc:
        with (
            tc.tile_pool(name="const", bufs=1) as const,
            tc.tile_pool(name="xchunk", bufs=5) as xchunk,
            tc.tile_pool(name="qr", bufs=8) as qrp,
            tc.tile_pool(name="qt", bufs=10) as qtp,
            tc.tile_pool(name="pp", bufs=4) as pp,
            tc.tile_pool(name="osb", bufs=2) as osb,
            tc.tile_pool(name="tri", bufs=2) as trip,
            tc.tile_pool(name="vec", bufs=6) as vec,
            tc.tile_pool(name="psA", bufs=2, space="PSUM") as psA,
            tc.tile_pool(name="psim", bufs=4, space="PSUM") as psimp,
            tc.tile_pool(name="pfeat", bufs=2, space="PSUM") as pfeat,
        ):
            def copyback(dst, src):
                if _flip[0] % 2 == 0:
                    nc.scalar.copy(dst, src)
                else:
                    nc.vector.tensor_copy(dst, src)
                _flip[0] += 1

            # batch-0 first-wave loads go first so DMA starts during preamble
            pre_x = {}
            for mi in range(CB):
                xt = xchunk.tile([P, 1024], f32, tag="xc", name=f"prex{mi}")
                nc.sync.dma_start(
                    out=xt[:, :512], in_=x_ext[0, mi * P:(mi + 1) * P, 0:512]
                )
                pre_x[mi] = xt

            ident_f = const.tile([P, P], f32)
            make_identity(nc, ident_f)
            ident_r = const.tile([P, P], f32r)
            nc.vector.tensor_copy(ident_r[:], ident_f[:])
            gamma_sb = const.tile([P, 1], f32)
            nc.sync.dma_start(out=gamma_sb[:], in_=g_ext[:].to_broadcast([P, 1]))

            # dummy matmuls while the first loads land: warms the PE clock
            # gate (HAM) so real matmuls start at full rate
            warm = psA.tile([P, C], f32, tag="psA", name="warmup")
            for i in range(8):
                nc.tensor.matmul(warm[:, :P], ident_f[:], ident_f[:],
                                 start=True, stop=True)

            # column waves per batch; first two finer to cut startup latency
            WAVES = [(0, 512), (512, 512), (1024, 1024), (2048, 1024), (3072, 1024)]
            C0S = [min(mi * P, 2 * P) for mi in range(CB)]  # 0,128,256,256

            def phase1(b):
                """load+round x, transpose to qT, sim matmuls (upper-tri)."""
                st = {}
                st["qr"] = [qrp.tile([P, HW], f32r, tag="qr", name=f"qr{b}_{i}")
                            for i in range(CB)]
                st["psim"] = [psimp.tile([P, C], f32, tag="psim",
                                         name=f"psim{b}_{i}") for i in range(CB)]
                qr_t, psim = st["qr"], st["psim"]
                qt_tiles = {}

                def mm1(kn):
                    for mi in range(CB):
                        c0 = C0S[mi]
                        nc.tensor.matmul(
                            psim[mi][:, c0:],
                            qt_tiles[kn][:, mi * P:(mi + 1) * P],
                            qt_tiles[kn][:, c0:],
                            start=(kn == 0),
                            stop=(kn == KN - 1),
                        )

                pending = []
                for (w0, wlen) in WAVES:
                    for mi in range(CB):
                        if b == 0 and w0 == 0:
                            xt = pre_x[mi]
                        else:
                            xt = xchunk.tile([P, 1024], f32, tag="xc")
                            nc.sync.dma_start(
                                out=xt[:, :wlen],
                                in_=x_ext[b, mi * P:(mi + 1) * P, w0:w0 + wlen],
                            )
                        nc.vector.tensor_copy(
                            qr_t[mi][:, w0:w0 + wlen], xt[:, :wlen]
                        )
                    for kq in range(wlen // P):
                        kn = w0 // P + kq
                        pst = psA.tile([P, C], f32r, tag="psA")
                        for ci in range(CB):
                            nc.tensor.transpose(
                                pst[:, ci * P:(ci + 1) * P],
                                qr_t[ci][:, kn * P:(kn + 1) * P],
                                ident_r[:],
                            )
                        qt = qtp.tile([P, C], f32r, tag="qt", name=f"qt{b}_{kn}")
                        qt_tiles[kn] = qt
                        copyback(qt[:], pst[:])
                        pending.append(kn)
                        if len(pending) > 2:
                            mm1(pending.pop(0))
                for kn in pending:
                    mm1(kn)
                return st

            def softmax_pt(b, st):
                """tri fills, rowwise softmax, build lhsT = T(p*gamma/Z)+I."""
                psim = st["psim"]
                for (i, j) in [(1, 0), (2, 0), (2, 1), (3, 0), (3, 1)]:
                    tmp = trip.tile([P, P], f32, tag="tri")
                    nc.scalar.copy(tmp[:], psim[j][:, i * P:(i + 1) * P])
                    nc.tensor.transpose(
                        psim[i][:, j * P:(j + 1) * P], tmp[:], ident_f[:]
                    )
                ps_t = []
                for mi in range(CB):
                    mrow = vec.tile([P, 1], f32, tag="mrow")
                    nc.vector.tensor_reduce(
                        mrow[:], psim[mi][:], axis=AX.X, op=ALU.min
                    )
                    zrow = vec.tile([P, 1], f32, tag="zrow")
                    p_t = pp.tile([P, C], f32r, tag="p", bufs=2)
                    nc.scalar.activation(
                        p_t[:], psim[mi][:], ACTF.Exp,
                        bias=mrow[:], scale=-1.0, accum_out=zrow[:],
                    )
                    rz = vec.tile([P, 1], f32, tag="rz")
                    nc.vector.reciprocal(rz[:], zrow[:])
                    rzg = vec.tile([P, 1], f32, tag="rzg")
                    nc.vector.tensor_mul(rzg[:], rz[:], gamma_sb[:])
                    p_s = pp.tile([P, C], f32r, tag="psc", bufs=4)
                    nc.vector.tensor_scalar_mul(p_s[:], p_t[:], rzg[:])
                    ps_t.append(p_s)
                pt_t = []
                for kd in range(CB):
                    pst = pfeat.tile([P, C], f32r, tag="pf")
                    for ci in range(CB):
                        nc.tensor.transpose(
                            pst[:, ci * P:(ci + 1) * P],
                            ps_t[ci][:, kd * P:(kd + 1) * P],
                            ident_r[:],
                        )
                    t = pp.tile([P, C], f32r, tag="pt")
                    copyback(t[:], pst[:])
                    nc.vector.tensor_add(
                        t[:, kd * P:(kd + 1) * P],
                        t[:, kd * P:(kd + 1) * P],
                        ident_r[:],
                    )
                    pt_t.append(t)
                st["pt"] = pt_t

            def mm2(b, st):
                """out = (gamma*diag(1/Z)*P + I) @ q, staged stores."""
                qr_t, pt_t = st["qr"], st["pt"]
                for mi in range(CB):
                    fine = (b == NB - 1 and mi == CB - 1)
                    for half in range(2):
                        stg = osb.tile([P, HW // 2], f32, tag="ot")
                        for njh in range(NJ // 2):
                            nj = half * (NJ // 2) + njh
                            pf = pfeat.tile([P, 512], f32, tag="pf")
                            for kd in range(CB):
                                nc.tensor.matmul(
                                    pf[:],
                                    pt_t[kd][:, mi * P:(mi + 1) * P],
                                    qr_t[kd][:, nj * 512:(nj + 1) * 512],
                                    start=(kd == 0),
                                    stop=(kd == CB - 1),
                                )
                            copyback(stg[:, njh * 512:(njh + 1) * 512], pf[:])
                            if fine:
                                nc.sync.dma_start(
                                    out=o_ext[b, mi * P:(mi + 1) * P,
                                              nj * 512:(nj + 1) * 512],
                                    in_=stg[:, njh * 512:(njh + 1) * 512],
                                )
                        if not fine:
                            nc.sync.dma_start(
                                out=o_ext[b, mi * P:(mi + 1) * P,
                                          half * (HW // 2):(half + 1) * (HW // 2)],
                                in_=stg[:],
                            )

            # phase-reordered emission: batch-1 phase 1 is emitted BEFORE
            # batch-0's mm2, so mm2(0) (lower priority) becomes the PE
            # filler for batch-1's softmax dependency chain, and batch-1
            # finishes earlier. Sync queue order is then loads(0), loads(1),
            # stores(0), stores(1): stores never block loads.
            st0 = phase1(0)
            softmax_pt(0, st0)
            st1 = phase1(1)
            mm2(0, st0)
            softmax_pt(1, st1)
            mm2(1, st1)

    nc.finalize()
    return nc


def get_bass():
    if "nc" not in _BUILD_CACHE:
        _BUILD_CACHE["nc"] = build_bass()
    return _BUILD_CACHE["nc"]


def make_in_maps(x, gamma):
    x = np.ascontiguousarray(np.asarray(x, dtype=np.float32)).reshape(B, C, HW)
    gamma = np.asarray(gamma, dtype=np.float32).reshape(1)
    return [
        {"x": x[i * NB:(i + 1) * NB], "gamma": gamma}
        for i in range(NCORES)
    ]


def run(x, gamma, trace=False, **trace_kwargs):
    from concourse.bass_utils import run_bass_kernel_spmd

    nc = get_bass()
    res = run_bass_kernel_spmd(
        nc, make_in_maps(x, gamma), core_ids=list(range(NCORES)),
        trace=trace, **trace_kwargs,
    )
    out = np.concatenate([res.results[i]["out"] for i in range(NCORES)], axis=0)
    return out.reshape(B, C, H, W), res


def kernel(x, gamma):
    out, _ = run(x, gamma, trace=False)
    return out



# revision 5
# speedup vs baseline: 1.2590x; 1.2590x over previous
"""Channel-attention (CAM) Trainium2 kernel.

Reference computation (per batch b of 16):
    q   = x[b].reshape(C, HW)                  # C=512, HW=4096
    sim = q @ q.T                              # [C, C], symmetric
    attn = softmax(max(sim) - sim, axis=-1)    # == exp(min_r - sim) / Z_r
    out[b] = gamma * attn @ q + x[b]

Sharding: data-parallel over batch across 8 NeuronCores (2 batches/core).
kernel() takes full inputs, shards internally, returns the full output.

Per-core kernel design (v2, fp16 matmul path):
  - All matmuls in float16. fp16 streams the PE at 1 col/cycle @2.4GHz vs
    f32r's measured ~1.28 cyc/col, transposes at 1.0 vs 1.5 cyc/col, and
    16-bit stationaries get FWL (2x faster LDWEIGHTS). Accuracy: fp16
    mantissa gives sim errors ~0.04 (vs bf16's ~0.3 which flips softmax
    winners); CPU-simulated end-to-end rel_l2 = 7.8e-4, 25x under the
    2e-2 gate. PSUM accumulation stays fp32.
  - x is streamed in column-waves, cast f32->fp16 (DVE, 2x rate for
    16-bit), transposed on the PE (identity matmuls) into qT tiles [n, c];
    sim matmuls run two transpose-chunks behind, so DMA/cast/transpose/
    matmul pipeline.
  - sim is symmetric: compute the exact upper-tri block rows (cols >=
    (0,128,256,384)) and fill the 6 missing lower [128,128] blocks by
    PE-transposing their mirror blocks out of PSUM (via f32 SBUF bounce).
  - softmax via ACT: p = exp(min_r - sim) with accum_out producing Z in
    the same pass (fp16 out); rows scaled by gamma/Z (DVE), PE-transposed,
    and the identity is added to the diagonal so the second matmul directly
    computes gamma*attn@q + q = out (residual folded; x == q here).
  - mm2 results are staged [128, 2048] in SBUF and stored with 8KB
    descriptor lines, all on the Sync HWDGE queue.
  - phase-reordered emission: batch-1's load/transpose/sim phase is
    emitted BEFORE batch-0's second matmul, so mm2(0) becomes the
    lower-priority PE filler behind batch-1's dependency chains, and the
    Sync queue orders loads(0), loads(1), stores(0), stores(1).
  - dummy identity matmuls at t=0 pre-warm the PE clock gate (HAM)
    while the first loads are in flight.
"""
import sys

if "/opt/trn_rl_repo" not in sys.path:
    sys.path.insert(0, "/opt/trn_rl_repo")

import numpy as np

B, C, H, W = 16, 512, 64, 64
HW = H * W
NCORES = 8
NB = B // NCORES          # batches per core
P = 128
CB = C // P               # 4 channel blocks
KN = HW // P              # 32 contraction chunks for sim
NJ = HW // 512            # 8 output column chunks

_BUILD_CACHE = {}


def build_bass():
    import concourse.bacc as bacc
    import concourse.tile as tile
    from concourse import mybir
    from concourse.masks import make_identity

    f32 = mybir.dt.float32
    f16 = mybir.dt.float16
    AX = mybir.AxisListType
    ALU = mybir.AluOpType
    ACTF = mybir.ActivationFunctionType

    nc = bacc.Bacc()
    x_ext = nc.declare_dram_parameter("x", [NB, C, HW], f32, isOutput=False)
    g_ext = nc.declare_dram_parameter("gamma", [1], f32, isOutput=False)
    o_ext = nc.declare_dram_parameter("out", [NB, C, HW], f32, isOutput=True)

    # alternate PSUM->SBUF copies between ACT and DVE to balance engines
    _flip = [0]

    with tile.TileContext(nc) as tc:
        with (
            tc.tile_pool(name="const", bufs=1) as const,
            tc.tile_pool(name="xchunk", bufs=5) as xchunk,
            tc.tile_pool(name="qr", bufs=8) as qrp,
            tc.tile_pool(name="qt", bufs=10) as qtp,
            tc.tile_pool(name="pp", bufs=4) as pp,
            tc.tile_pool(name="osb", bufs=2) as osb,
            tc.tile_pool(name="tri", bufs=2) as trip,
            tc.tile_pool(name="vec", bufs=6) as vec,
            tc.tile_pool(name="psA", bufs=2, space="PSUM") as psA,
            tc.tile_pool(name="psim", bufs=4, space="PSUM") as psimp,
            tc.tile_pool(name="pfeat", bufs=2, space="PSUM") as pfeat,
        ):
            def copyback(dst, src):
                if _flip[0] % 2 == 0:
                    nc.scalar.copy(dst, src)
                else:
                    nc.vector.tensor_copy(dst, src)
                _flip[0] += 1

            # batch-0 first-wave loads go first so DMA starts during preamble
            pre_x = {}
            for mi in range(CB):
                xt = xchunk.tile([P, 1024], f32, tag="xc", name=f"prex{mi}")
                nc.sync.dma_start(
                    out=xt[:, :512], in_=x_ext[0, mi * P:(mi + 1) * P, 0:512]
                )
                pre_x[mi] = xt

            ident_f = const.tile([P, P], f32)
            make_identity(nc, ident_f)
            ident_h = const.tile([P, P], f16)
            nc.vector.tensor_copy(ident_h[:], ident_f[:])
            gamma_sb = const.tile([P, 1], f32)
            nc.sync.dma_start(out=gamma_sb[:], in_=g_ext[:].to_broadcast([P, 1]))

            # dummy matmuls while the first loads land: warms the PE clock
            # gate (HAM) so real matmuls start at full rate. Must be real
            # matmuls (f32 out) — transpose-mode doesn't count as PE-busy
            # for HAM. 28 x N=128 at cold rate ~= 3us ~= one SHORT window.
            warm = psA.tile([P, C], f32, tag="psA", name="warmup")
            for i in range(28):
                nc.tensor.matmul(warm[:, :P], ident_h[:], ident_h[:],
                                 start=True, stop=True)

            # column waves per batch; first two finer to cut startup latency
            WAVES = [(0, 512), (512, 512), (1024, 1024), (2048, 1024), (3072, 1024)]
            C0S = [mi * P for mi in range(CB)]  # 0,128,256,384 (exact upper tri)
            TRI = [(1, 0), (2, 0), (2, 1), (3, 0), (3, 1), (3, 2)]

            def phase1(b):
                """load+cast x, transpose to qT, sim matmuls (upper-tri)."""
                st = {}
                st["qr"] = [qrp.tile([P, HW], f16, tag="qr", name=f"qr{b}_{i}")
                            for i in range(CB)]
                st["psim"] = [psimp.tile([P, C], f32, tag="psim",
                                         name=f"psim{b}_{i}") for i in range(CB)]
                qr_t, psim = st["qr"], st["psim"]
                qt_tiles = {}

                def mm1(kn):
                    for mi in range(CB):
                        c0 = C0S[mi]
                        nc.tensor.matmul(
                            psim[mi][:, c0:],
                            qt_tiles[kn][:, mi * P:(mi + 1) * P],
                            qt_tiles[kn][:, c0:],
                            start=(kn == 0),
                            stop=(kn == KN - 1),
                        )

                pending = []
                for (w0, wlen) in WAVES:
                    for mi in range(CB):
                        if b == 0 and w0 == 0:
                            xt = pre_x[mi]
                        else:
                            xt = xchunk.tile([P, 1024], f32, tag="xc")
                            nc.sync.dma_start(
                                out=xt[:, :wlen],
                                in_=x_ext[b, mi * P:(mi + 1) * P, w0:w0 + wlen],
                            )
                        nc.vector.tensor_copy(
                            qr_t[mi][:, w0:w0 + wlen], xt[:, :wlen]
                        )
                    for kq in range(wlen // P):
                        kn = w0 // P + kq
                        pst = psA.tile([P, C], f16, tag="psA")
                        for ci in range(CB):
                            nc.tensor.transpose(
                                pst[:, ci * P:(ci + 1) * P],
                                qr_t[ci][:, kn * P:(kn + 1) * P],
                                ident_h[:],
                            )
                        qt = qtp.tile([P, C], f16, tag="qt", name=f"qt{b}_{kn}")
                        qt_tiles[kn] = qt
                        copyback(qt[:], pst[:])
                        pending.append(kn)
                        if len(pending) > 2:
                            mm1(pending.pop(0))
                for kn in pending:
                    mm1(kn)
                return st

            def softmax_pt(b, st):
                """tri fills, rowwise softmax, build lhsT = T(p*gamma/Z)+I."""
                psim = st["psim"]
                for (i, j) in TRI:
                    tmp = trip.tile([P, P], f32, tag="tri")
                    nc.scalar.copy(tmp[:], psim[j][:, i * P:(i + 1) * P])
                    nc.tensor.transpose(
                        psim[i][:, j * P:(j + 1) * P], tmp[:], ident_f[:]
                    )
                ps_t = []
                for mi in range(CB):
                    mrow = vec.tile([P, 1], f32, tag="mrow")
                    nc.vector.tensor_reduce(
                        mrow[:], psim[mi][:], axis=AX.X, op=ALU.min
                    )
                    zrow = vec.tile([P, 1], f32, tag="zrow")
                    p_t = pp.tile([P, C], f16, tag="p", bufs=2)
                    nc.scalar.activation(
                        p_t[:], psim[mi][:], ACTF.Exp,
                        bias=mrow[:], scale=-1.0, accum_out=zrow[:],
                    )
                    rz = vec.tile([P, 1], f32, tag="rz")
                    nc.vector.reciprocal(rz[:], zrow[:])
                    rzg = vec.tile([P, 1], f32, tag="rzg")
                    nc.vector.tensor_mul(rzg[:], rz[:], gamma_sb[:])
                    p_s = pp.tile([P, C], f16, tag="psc", bufs=4)
                    nc.vector.tensor_scalar_mul(p_s[:], p_t[:], rzg[:])
                    ps_t.append(p_s)
                pt_t = []
                for kd in range(CB):
                    pst = pfeat.tile([P, C], f16, tag="pf")
                    for ci in range(CB):
                        nc.tensor.transpose(
                            pst[:, ci * P:(ci + 1) * P],
                            ps_t[ci][:, kd * P:(kd + 1) * P],
                            ident_h[:],
                        )
                    t = pp.tile([P, C], f16, tag="pt")
                    copyback(t[:], pst[:])
                    nc.vector.tensor_add(
                        t[:, kd * P:(kd + 1) * P],
                        t[:, kd * P:(kd + 1) * P],
                        ident_h[:],
                    )
                    pt_t.append(t)
                st["pt"] = pt_t

            def mm2(b, st):
                """out = (gamma*diag(1/Z)*P + I) @ q, staged stores."""
                qr_t, pt_t = st["qr"], st["pt"]
                for mi in range(CB):
                    fine = (b == NB - 1 and mi == CB - 1)
                    for half in range(2):
                        stg = osb.tile([P, HW // 2], f32, tag="ot")
                        for njh in range(NJ // 2):
                            nj = half * (NJ // 2) + njh
                            pf = pfeat.tile([P, 512], f32, tag="pf")
                            for kd in range(CB):
                                nc.tensor.matmul(
                                    pf[:],
                                    pt_t[kd][:, mi * P:(mi + 1) * P],
                                    qr_t[kd][:, nj * 512:(nj + 1) * 512],
                                    start=(kd == 0),
                                    stop=(kd == CB - 1),
                                )
                            copyback(stg[:, njh * 512:(njh + 1) * 512], pf[:])
                            if fine:
                                nc.sync.dma_start(
                                    out=o_ext[b, mi * P:(mi + 1) * P,
                                              nj * 512:(nj + 1) * 512],
                                    in_=stg[:, njh * 512:(njh + 1) * 512],
                                )
                        if not fine:
                            nc.sync.dma_start(
                                out=o_ext[b, mi * P:(mi + 1) * P,
                                          half * (HW // 2):(half + 1) * (HW // 2)],
                                in_=stg[:],
                            )

            # phase-reordered emission: batch-1 phase 1 is emitted BEFORE
            # batch-0's mm2, so mm2(0) (lower priority) becomes the PE
            # filler for batch-1's softmax dependency chain, and batch-1
            # finishes earlier. Sync queue order is then loads(0), loads(1),
            # stores(0), stores(1): stores never block loads.
            st0 = phase1(0)
            softmax_pt(0, st0)
            st1 = phase1(1)
            mm2(0, st0)
            softmax_pt(1, st1)
            mm2(1, st1)

    nc.finalize()
    return nc


def get_bass():
    if "nc" not in _BUILD_CACHE:
        _BUILD_CACHE["nc"] = build_bass()
    return _BUILD_CACHE["nc"]


def make_in_maps(x, gamma):
    x = np.ascontiguousarray(np.asarray(x, dtype=np.float32)).reshape(B, C, HW)
    gamma = np.asarray(gamma, dtype=np.float32).reshape(1)
    return [
        {"x": x[i * NB:(i + 1) * NB], "gamma": gamma}
        for i in range(NCORES)
    ]


def run(x, gamma, trace=False, **trace_kwargs):
    from concourse.bass_utils import run_bass_kernel_spmd

    nc = get_bass()
    res = run_bass_kernel_spmd(
        nc, make_in_maps(x, gamma), core_ids=list(range(NCORES)),
        trace=trace, **trace_kwargs,
    )
    out = np.concatenate([res.results[i]["out"] for i in range(NCORES)], axis=0)
    return out.reshape(B, C, H, W), res


def kernel(x, gamma):
    out, _ = run(x, gamma, trace=False)
    return out


# revision 6
# speedup vs baseline: 1.3214x; 1.0495x over previous
"""Channel-attention (CAM) Trainium2 kernel.

Reference computation (per batch b of 16):
    q   = x[b].reshape(C, HW)                  # C=512, HW=4096
    sim = q @ q.T                              # [C, C], symmetric
    attn = softmax(max(sim) - sim, axis=-1)    # == exp(min_r - sim) / Z_r
    out[b] = gamma * attn @ q + x[b]

Sharding: data-parallel over batch across 8 NeuronCores (2 batches/core).
kernel() takes full inputs, shards internally, returns the full output.

Per-core kernel design (v3, fp16 matmul path):
  - All matmuls in float16: fp16 streams the PE at 1 col/cycle @2.4GHz
    (vs f32r ~1.28), FWL halves LDWEIGHTS, casts/copybacks run at 2x DVE
    rate. PSUM accumulation stays fp32. CPU-simulated end-to-end rel_l2
    = 7.8e-4 (gate 2e-2); bf16 would flip softmax winners (sim err ~0.3).
  - ALL transposes are REGULAR matmuls (data stationary, identity moving)
    instead of transpose-mode: same cost (LS 128 + stream 128), but they
    count as PE-busy for the HAM clock gate, so the 1.2GHz re-throttle
    oscillation seen with transpose-mode during phase1/softmax is gone.
  - x is streamed in column-waves, cast f32->fp16 (DVE), transposed on
    the PE into qT tiles [n, c]; sim matmuls run two transpose-chunks
    behind, so DMA/cast/transpose/matmul pipeline.
  - sim is symmetric: compute the exact upper-tri block rows (cols >=
    (0,128,256,384)); the 6 missing lower blocks are filled by cast-to-
    fp16 + matmul-transpose out of the mirror blocks, interleaved per-mi
    with the softmax reduce/exp chain to shorten the serial boundary.
  - softmax via ACT: p = exp(min_r - sim) with accum_out producing Z in
    the same pass (fp16 out); rows scaled by gamma/Z (DVE), PE-transposed,
    identity added so mm2 computes gamma*attn@q + q = out directly.
  - batch-1's first two column waves (loads + casts) are prefetched
    before batch-0's softmax, so the DVE queue has no head-of-line block
    and batch-1 transposes fill the PE during batch-0's softmax chain.
  - mm2 stores: batch 0 staged [128, 2048] (8KB lines); batch 1 stored
    per-512 as each PSUM block drains, to shorten the tail.
  - 6 real warmup matmuls pre-warm the HAM clock gate during preamble.
"""
import sys

if "/opt/trn_rl_repo" not in sys.path:
    sys.path.insert(0, "/opt/trn_rl_repo")

import numpy as np

B, C, H, W = 16, 512, 64, 64
HW = H * W
NCORES = 8
NB = B // NCORES          # batches per core
P = 128
CB = C // P               # 4 channel blocks
KN = HW // P              # 32 contraction chunks for sim
NJ = HW // 512            # 8 output column chunks

_BUILD_CACHE = {}


def build_bass():
    import concourse.bacc as bacc
    import concourse.tile as tile
    from concourse import mybir
    from concourse.masks import make_identity

    f32 = mybir.dt.float32
    f16 = mybir.dt.float16
    AX = mybir.AxisListType
    ALU = mybir.AluOpType
    ACTF = mybir.ActivationFunctionType

    nc = bacc.Bacc()
    x_ext = nc.declare_dram_parameter("x", [NB, C, HW], f32, isOutput=False)
    g_ext = nc.declare_dram_parameter("gamma", [1], f32, isOutput=False)
    o_ext = nc.declare_dram_parameter("out", [NB, C, HW], f32, isOutput=True)

    # alternate PSUM->SBUF copies between ACT and DVE to balance engines
    _flip = [0]

    with tile.TileContext(nc) as tc:
        with (
            tc.tile_pool(name="const", bufs=1) as const,
            tc.tile_pool(name="xchunk", bufs=6) as xchunk,
            tc.tile_pool(name="qr", bufs=8) as qrp,
            tc.tile_pool(name="qt", bufs=10) as qtp,
            tc.tile_pool(name="pp", bufs=4) as pp,
            tc.tile_pool(name="osb", bufs=4) as osb,
            tc.tile_pool(name="tri", bufs=2) as trip,
            tc.tile_pool(name="vec", bufs=6) as vec,
            tc.tile_pool(name="psA", bufs=2, space="PSUM") as psA,
            tc.tile_pool(name="psim", bufs=4, space="PSUM") as psimp,
            tc.tile_pool(name="pfeat", bufs=2, space="PSUM") as pfeat,
        ):
            def copyback(dst, src):
                if _flip[0] % 2 == 0:
                    nc.scalar.copy(dst, src)
                else:
                    nc.vector.tensor_copy(dst, src)
                _flip[0] += 1

            # batch-0 first-wave loads go first so DMA starts during preamble
            pre_x = {}
            for mi in range(CB):
                xt = xchunk.tile([P, 1024], f32, tag="xc", name=f"prex{mi}")
                nc.sync.dma_start(
                    out=xt[:, :512], in_=x_ext[0, mi * P:(mi + 1) * P, 0:512]
                )
                pre_x[mi] = xt

            ident_f = const.tile([P, P], f32)
            make_identity(nc, ident_f)
            ident_h = const.tile([P, P], f16)
            nc.vector.tensor_copy(ident_h[:], ident_f[:])
            gamma_sb = const.tile([P, 1], f32)
            nc.sync.dma_start(out=gamma_sb[:], in_=g_ext[:].to_broadcast([P, 1]))

            # mm_transpose: out[P,128](f32 PSUM) = in_[P,128](f16).T via a
            # REGULAR matmul with identity moving. Counts as HAM PE-busy
            # (transpose-mode does not), same cost.
            def mm_transpose(out, in_):
                nc.tensor.matmul(out, in_, ident_h[:], start=True, stop=True)

            # real warmup matmuls while the first loads land (HAM warm-up)
            warm = psA.tile([P, C], f32, tag="psA", name="warmup")
            for i in range(6):
                nc.tensor.matmul(warm[:, :P], ident_h[:], ident_h[:],
                                 start=True, stop=True)

            # column waves per batch; first two finer to cut startup latency
            WAVES = [(0, 512), (512, 512), (1024, 1024), (2048, 1024), (3072, 1024)]
            C0S = [mi * P for mi in range(CB)]  # 0,128,256,384 (exact upper tri)
            # tri fills grouped by destination block-row
            TRI = {1: [(1, 0)], 2: [(2, 0), (2, 1)], 3: [(3, 0), (3, 1), (3, 2)]}

            def alloc_state(b):
                st = {}
                st["qr"] = [qrp.tile([P, HW], f16, tag="qr", name=f"qr{b}_{i}")
                            for i in range(CB)]
                st["done_waves"] = set()
                return st

            def load_wave(b, st, w0, wlen, use_pre=False):
                """DMA + cast one column wave of batch b into qr tiles."""
                for mi in range(CB):
                    if use_pre:
                        xt = pre_x[mi]
                    else:
                        xt = xchunk.tile([P, 1024], f32, tag="xc")
                        nc.sync.dma_start(
                            out=xt[:, :wlen],
                            in_=x_ext[b, mi * P:(mi + 1) * P, w0:w0 + wlen],
                        )
                    nc.vector.tensor_copy(
                        st["qr"][mi][:, w0:w0 + wlen], xt[:, :wlen]
                    )
                st["done_waves"].add(w0)

            def phase1(b, st):
                """transpose to qT, sim matmuls (upper-tri)."""
                st["psim"] = [psimp.tile([P, C], f32, tag="psim",
                                         name=f"psim{b}_{i}") for i in range(CB)]
                qr_t, psim = st["qr"], st["psim"]
                qt_tiles = {}

                def mm1(kn):
                    for mi in range(CB):
                        c0 = C0S[mi]
                        nc.tensor.matmul(
                            psim[mi][:, c0:],
                            qt_tiles[kn][:, mi * P:(mi + 1) * P],
                            qt_tiles[kn][:, c0:],
                            start=(kn == 0),
                            stop=(kn == KN - 1),
                        )

                pending = []
                for (w0, wlen) in WAVES:
                    if w0 not in st["done_waves"]:
                        load_wave(b, st, w0, wlen,
                                  use_pre=(b == 0 and w0 == 0))
                    for kq in range(wlen // P):
                        kn = w0 // P + kq
                        pst = psA.tile([P, C], f32, tag="psA")
                        for ci in range(CB):
                            mm_transpose(
                                pst[:, ci * P:(ci + 1) * P],
                                qr_t[ci][:, kn * P:(kn + 1) * P],
                            )
                        qt = qtp.tile([P, C], f16, tag="qt", name=f"qt{b}_{kn}")
                        qt_tiles[kn] = qt
                        copyback(qt[:], pst[:])
                        pending.append(kn)
                        if len(pending) > 2:
                            mm1(pending.pop(0))
                for kn in pending:
                    mm1(kn)
                return st

            def softmax_pt(b, st):
                """tri fills + rowwise softmax (pipelined per block-row),
                then build lhsT = T(p*gamma/Z)+I."""
                psim = st["psim"]
                # emit tri fills for row mi, then its reduce/exp chain, so
                # the ACT/DVE/PE work of successive rows pipelines
                ps_t = []
                for mi in range(CB):
                    for (i, j) in TRI.get(mi, []):
                        tmp = trip.tile([P, P], f16, tag="tri")
                        nc.scalar.copy(tmp[:], psim[j][:, i * P:(i + 1) * P])
                        mm_transpose(psim[i][:, j * P:(j + 1) * P], tmp[:])
                    mrow = vec.tile([P, 1], f32, tag="mrow")
                    nc.vector.tensor_reduce(
                        mrow[:], psim[mi][:], axis=AX.X, op=ALU.min
                    )
                    zrow = vec.tile([P, 1], f32, tag="zrow")
                    p_t = pp.tile([P, C], f16, tag="p", bufs=2)
                    nc.scalar.activation(
                        p_t[:], psim[mi][:], ACTF.Exp,
                        bias=mrow[:], scale=-1.0, accum_out=zrow[:],
                    )
                    rz = vec.tile([P, 1], f32, tag="rz")
                    nc.vector.reciprocal(rz[:], zrow[:])
                    rzg = vec.tile([P, 1], f32, tag="rzg")
                    nc.vector.tensor_mul(rzg[:], rz[:], gamma_sb[:])
                    p_s = pp.tile([P, C], f16, tag="psc", bufs=4)
                    nc.vector.tensor_scalar_mul(p_s[:], p_t[:], rzg[:])
                    ps_t.append(p_s)
                pt_t = []
                for kd in range(CB):
                    pst = pfeat.tile([P, C], f32, tag="pf")
                    for ci in range(CB):
                        mm_transpose(
                            pst[:, ci * P:(ci + 1) * P],
                            ps_t[ci][:, kd * P:(kd + 1) * P],
                        )
                    t = pp.tile([P, C], f16, tag="pt")
                    copyback(t[:], pst[:])
                    nc.vector.tensor_add(
                        t[:, kd * P:(kd + 1) * P],
                        t[:, kd * P:(kd + 1) * P],
                        ident_h[:],
                    )
                    pt_t.append(t)
                st["pt"] = pt_t

            def mm2(b, st):
                """out = (gamma*diag(1/Z)*P + I) @ q, staged stores."""
                qr_t, pt_t = st["qr"], st["pt"]
                fine = (b == NB - 1)
                for mi in range(CB):
                    if fine:
                        # store per-512 as each PSUM block drains (short tail)
                        for nj in range(NJ):
                            pf = pfeat.tile([P, 512], f32, tag="pf")
                            for kd in range(CB):
                                nc.tensor.matmul(
                                    pf[:],
                                    pt_t[kd][:, mi * P:(mi + 1) * P],
                                    qr_t[kd][:, nj * 512:(nj + 1) * 512],
                                    start=(kd == 0),
                                    stop=(kd == CB - 1),
                                )
                            stg = osb.tile([P, 512], f32, tag="otf")
                            copyback(stg[:], pf[:])
                            nc.sync.dma_start(
                                out=o_ext[b, mi * P:(mi + 1) * P,
                                          nj * 512:(nj + 1) * 512],
                                in_=stg[:],
                            )
                    else:
                        for half in range(2):
                            stg = osb.tile([P, HW // 2], f32, tag="ot")
                            for njh in range(NJ // 2):
                                nj = half * (NJ // 2) + njh
                                pf = pfeat.tile([P, 512], f32, tag="pf")
                                for kd in range(CB):
                                    nc.tensor.matmul(
                                        pf[:],
                                        pt_t[kd][:, mi * P:(mi + 1) * P],
                                        qr_t[kd][:, nj * 512:(nj + 1) * 512],
                                        start=(kd == 0),
                                        stop=(kd == CB - 1),
                                    )
                                copyback(stg[:, njh * 512:(njh + 1) * 512],
                                         pf[:])
                            nc.sync.dma_start(
                                out=o_ext[b, mi * P:(mi + 1) * P,
                                          half * (HW // 2):(half + 1) * (HW // 2)],
                                in_=stg[:],
                            )

            # phase-reordered emission: batch-1's first waves (loads+casts)
            # are prefetched before batch-0's softmax so the DVE queue has
            # no head-of-line block and batch-1 transposes (real matmuls)
            # fill the PE during batch-0's softmax chain. mm2(0) is emitted
            # after phase1(1) as the lower-priority PE filler.
            st0 = alloc_state(0)
            phase1(0, st0)
            st1 = alloc_state(1)
            load_wave(1, st1, 0, 512)
            load_wave(1, st1, 512, 512)
            softmax_pt(0, st0)
            phase1(1, st1)
            mm2(0, st0)
            softmax_pt(1, st1)
            mm2(1, st1)

    nc.finalize()
    return nc


def get_bass():
    if "nc" not in _BUILD_CACHE:
        _BUILD_CACHE["nc"] = build_bass()
    return _BUILD_CACHE["nc"]


def make_in_maps(x, gamma):
    x = np.ascontiguousarray(np.asarray(x, dtype=np.float32)).reshape(B, C, HW)
    gamma = np.asarray(gamma, dtype=np.float32).reshape(1)
    return [
        {"x": x[i * NB:(i + 1) * NB], "gamma": gamma}
        for i in range(NCORES)
    ]


def run(x, gamma, trace=False, **trace_kwargs):
    from concourse.bass_utils import run_bass_kernel_spmd

    nc = get_bass()
    res = run_bass_kernel_spmd(
        nc, make_in_maps(x, gamma), core_ids=list(range(NCORES)),
        trace=trace, **trace_kwargs,
    )
    out = np.concatenate([res.results[i]["out"] for i in range(NCORES)], axis=0)
    return out.reshape(B, C, H, W), res


def kernel(x, gamma):
    out, _ = run(x, gamma, trace=False)
    return out
